# revision 11
# baseline (speedup 1.0000x reference)
"""MultiHeadEMABlock Trainium2 kernel (8-core SPMD, bass/Tile).

Math (reference):
  h = LayerNorm_c(x[b,c,n] over c) * gamma + beta          (per (b,n))
  xe[b,n,h,d] = h[b,n,d] * expansion[h,d]
  y = causal damped EMA along n: y[t] = a_h*sum_{s<=t} q_h^{t-s} xe[s]
  out[b,d,n] = sum_h y[b,n,h,d]*reduction[h,d] + x

Key identities used:
  - The per-(h,d) scales commute with the EMA (EMA mixes along n only):
      out = x + sum_h rho_h[d] * S_h[d,n],  rho_h[d] = a_h*e[h,d]*r[h,d]*gamma[d]
      S_h = scan along n:  s_t = q_h * s_{t-1} + z_t   (z = normalized x)
    so all heads scan the SAME normalized input; no per-head input prep.
  - beta contributes a data-independent low-rank term added on host (exact).

Sharding: 8 cores = 4 batches x 2 sequence halves. Each core processes its
half plus a W-column halo from the left (zero-padded for the first half);
q^W underflows, so results are exact without any cross-core collective.

Layout: channels-major [c(partitions) x n(free)] end to end; the EMA runs as
DVE/GpSimd tensor_tensor_scan along the free axis; LayerNorm stats come from
replicated ones-matmuls on PE (partition-dim reduction).
"""
import contextlib
import ctypes
import sys
import types

import numpy as np

for _p in ("/root/.axon_site/_ro/trn_rl_repo", "/opt/trn_rl_repo"):
    if _p not in sys.path:
        sys.path.append(_p)

B, C, N, H = 4, 512, 4096, 8
EPS = 1e-5
N_CORES = 8
NHALF = N // 2
CT = C // 128  # channel tiles


# ---------------------------------------------------------------------------
# axon NTFF shim (lets run_bass_kernel_spmd(trace=True) capture HW profiles)
# ---------------------------------------------------------------------------
def _install_ntff_shim():
    if "antenv.axon_hooks" in sys.modules:
        return
    holder = {"hook": None}

    def _make(so_path):
        try:
            lib = ctypes.CDLL(so_path)
        except OSError:
            return None
        if not hasattr(lib, "axon_start_nrt_profile"):
            return None
        lib.axon_start_nrt_profile.argtypes = [
            ctypes.POINTER(ctypes.c_int64),
            ctypes.c_size_t,
        ]
        lib.axon_start_nrt_profile.restype = ctypes.c_int64
        lib.axon_stop_nrt_profile.argtypes = [ctypes.c_char_p]
        lib.axon_stop_nrt_profile.restype = ctypes.c_int64

        @contextlib.contextmanager
        def _hook(output_dir, device_ids):
            import jax

            jax.devices()
            if device_ids:
                ids = (ctypes.c_int64 * len(device_ids))(*device_ids)
                rc = lib.axon_start_nrt_profile(ids, len(device_ids))
            else:
                rc = lib.axon_start_nrt_profile(None, 0)
            if rc != 0:
                raise RuntimeError(f"axon_start_nrt_profile rc={rc}")
            try:
                yield
            finally:
                n = lib.axon_stop_nrt_profile(str(output_dir).encode())
                print(f"ntff profile: {n} file(s) -> {output_dir}", file=sys.stderr)

        return _hook

    mod = types.ModuleType("antenv.axon_hooks")
    mod.set_axon_ntff_profile_hook = lambda h: holder.__setitem__("hook", h)
    mod.get_axon_ntff_profile_hook = lambda: holder["hook"]
    sys.modules["antenv.axon_hooks"] = mod
    try:
        import antenv

        antenv.axon_hooks = mod
    except ImportError:
        pass
    holder["hook"] = _make("/opt/axon/libaxon_pjrt.so")


def _split_multiwait(nc, max_waits=1):
    """This walrus build rejects >1 sync wait per instruction; split extras
    onto same-engine NoOps inserted just before (per-engine order is the
    execution order, so semantics are preserved)."""
    from concourse import mybir

    k = [0]
    for fn in nc.m.functions:
        for blk in fn.blocks:
            out = []
            for inst in blk.instructions:
                si = getattr(inst, "sync_info", None)
                if si is not None and len(si.on_wait) > max_waits:
                    waits = list(si.on_wait)
                    for w in waits[max_waits:]:
                        k[0] += 1
                        out.append(
                            mybir.InstNoOp(
                                name=f"{inst.name}-mw{k[0]}",
                                sync_info=mybir.SyncInfo(on_wait=[w], on_update=[]),
                                bass_nofuse=True,
                                engine=inst.engine,
                            )
                        )
                    inst.sync_info = mybir.SyncInfo(
                        on_wait=waits[:max_waits], on_update=list(si.on_update)
                    )
                out.append(inst)
            blk.instructions[:] = out


# ---------------------------------------------------------------------------
# program builder
# ---------------------------------------------------------------------------
def build_program(qvals, W):
    """Build the SPMD per-core program. qvals: [H] f32 decay factors.
    W: halo width (multiple of 512). Returns nc."""
    import concourse.bass as bass
    import concourse.tile as tile
    from concourse import mybir

    NW = NHALF + W
    NK = NW // 512  # 512-wide stat chunks
    f32 = mybir.dt.float32
    bf16 = mybir.dt.bfloat16
    Op = mybir.AluOpType
    Act = mybir.ActivationFunctionType

    nc = bass.Bass(
        "TRN2",
        target_bir_lowering=False,
        debug=False,
        enable_asserts=False,
        num_devices=N_CORES,
    )
    xs_d = nc.dram_tensor("xs", [C, NW], f32, kind="ExternalInput").ap()
    rho_d = nc.dram_tensor("rho", [128, CT * H], f32, kind="ExternalInput").ap()
    out_d = nc.dram_tensor("out_t", [C, NHALF], f32, kind="ExternalOutput").ap()

    with tile.TileContext(nc) as tc:
        with contextlib.ExitStack() as ctx:
            pers = ctx.enter_context(tc.tile_pool(name="pers", bufs=1))
            xs_pool = ctx.enter_context(tc.tile_pool(name="xsp", bufs=2))
            sq_pool = ctx.enter_context(tc.tile_pool(name="sqp", bufs=4))
            ps_pool = ctx.enter_context(tc.tile_pool(name="ps", bufs=4, space="PSUM"))
            st_pool = ctx.enter_context(tc.tile_pool(name="stats", bufs=2))
            y_pool = ctx.enter_context(tc.tile_pool(name="yp", bufs=8))
            acc_pool = ctx.enter_context(tc.tile_pool(name="accp", bufs=5))
            out_pool = ctx.enter_context(tc.tile_pool(name="outp", bufs=2))

            # constants / params
            rho_sb = pers.tile([128, CT * H], f32, tag="rho")
            nc.sync.dma_start(out=rho_sb[:], in_=rho_d)
            ones = pers.tile([128, 128], bf16, tag="ones")
            nc.gpsimd.memset(ones[:], 1.0 / C)
            epsb = pers.tile([128, 1], f32, tag="eps")
            nc.gpsimd.memset(epsb[:], EPS)
            qcol = pers.tile([128, H], bf16, tag="qcol")
            for h in range(H):
                nc.gpsimd.memset(qcol[:, h : h + 1], float(qvals[h]))
            qbc = [qcol[:, h : h + 1].to_broadcast([128, NW]) for h in range(H)]

            # load + cast + square
            xb = [pers.tile([128, NW], bf16, tag=f"xb{ct}", name=f"xb{ct}") for ct in range(CT)]
            xsq = []
            for ct in range(CT):
                xst = xs_pool.tile([128, NW], f32, tag="xs")
                nc.sync.dma_start(out=xst[:], in_=xs_d[ct * 128 : (ct + 1) * 128, :])
                nc.scalar.activation(out=xb[ct][:], in_=xst[:], func=Act.Copy)
                sqt = sq_pool.tile([128, NW], bf16, tag="sq")
                nc.scalar.square(out=sqt[:], in_=xst[:])
                xsq.append(sqt)

            # layernorm stats via replicated ones-matmuls, then z = (x-m)*rstd
            z = [pers.tile([128, NW], bf16, tag=f"z{ct}", name=f"zt{ct}") for ct in range(CT)]
            for nk in range(NK):
                sl = slice(nk * 512, (nk + 1) * 512)
                ps_m = ps_pool.tile([128, 512], f32, tag="psm")
                ps_s = ps_pool.tile([128, 512], f32, tag="pss")
                for ct in range(CT):
                    nc.tensor.matmul(
                        out=ps_m[:], lhsT=ones[:], rhs=xb[ct][:, sl],
                        start=(ct == 0), stop=(ct == CT - 1),
                    )
                for ct in range(CT):
                    nc.tensor.matmul(
                        out=ps_s[:], lhsT=ones[:], rhs=xsq[ct][:, sl],
                        start=(ct == 0), stop=(ct == CT - 1),
                    )
                mean_bf = st_pool.tile([128, 512], bf16, tag="meanbf")
                nc.scalar.activation(out=mean_bf[:], in_=ps_m[:], func=Act.Copy)
                m2 = st_pool.tile([128, 512], f32, tag="m2")
                nc.scalar.square(out=m2[:], in_=ps_m[:])
                var = st_pool.tile([128, 512], f32, tag="var")
                nc.vector.scalar_tensor_tensor(
                    out=var[:], in0=ps_s[:], scalar=0.0, in1=m2[:],
                    op0=Op.bypass, op1=Op.subtract,
                )
                lnv = st_pool.tile([128, 512], f32, tag="lnv")
                nc.scalar.activation(out=lnv[:], in_=var[:], func=Act.Ln, bias=epsb[:])
                rstd = st_pool.tile([128, 512], bf16, tag="rstd")
                nc.scalar.activation(out=rstd[:], in_=lnv[:], func=Act.Exp, scale=-0.5)
                for ct in range(CT):
                    t = st_pool.tile([128, 512], bf16, tag="tnorm")
                    nc.vector.tensor_tensor(
                        out=t[:], in0=xb[ct][:, sl], in1=mean_bf[:], op=Op.subtract
                    )
                    nc.vector.tensor_tensor(
                        out=z[ct][:, sl], in0=t[:], in1=rstd[:], op=Op.mult
                    )

            # scans (DVE only — this walrus rejects scan/stt on Pool) and
            # per-ct combine chains on GpSimd (ts+tt pairs)
            for ct in range(CT):
                ys = []
                for h in range(H):
                    y = y_pool.tile([128, NW], bf16, tag="y")
                    nc.vector.tensor_tensor_scan(
                        out=y[:], data0=qbc[h], data1=z[ct][:],
                        initial=0.0, op0=Op.mult, op1=Op.add,
                    )
                    ys.append(y)
                acc = None
                for h in range(H):
                    ridx = ct * H + h
                    last = h == H - 1
                    scaled = acc_pool.tile([128, NHALF], bf16, tag="scl")
                    nc.gpsimd.tensor_scalar(
                        out=scaled[:], in0=ys[h][:, W:NW],
                        scalar1=rho_sb[:, ridx : ridx + 1], scalar2=None,
                        op0=Op.mult,
                    )
                    in1 = xb[ct][:, W:NW] if h == 0 else acc[:]
                    if last:
                        nxt = out_pool.tile([128, NHALF], f32, tag="out")
                    else:
                        nxt = acc_pool.tile([128, NHALF], bf16, tag="acc")
                    nc.gpsimd.tensor_tensor(
                        out=nxt[:], in0=scaled[:], in1=in1, op=Op.add
                    )
                    acc = nxt
                nc.sync.dma_start(
                    out=out_d[ct * 128 : (ct + 1) * 128, :], in_=acc[:]
                )
    return nc


def _host_params(ln_gamma, ln_beta, expansion, reduction, alphas, dampen_factors):
    a = 1.0 / (1.0 + np.exp(-alphas.astype(np.float64)))
    q = (1.0 - a) / (1.0 + np.exp(-dampen_factors.astype(np.float64)))
    qmax = float(q.max())
    W = 512
    while qmax**W > 1e-7 and W < NHALF:
        W += 512
    # rho[h,d] = a_h * e[h,d] * r[h,d] * gamma[d]
    rho = (
        a[:, None]
        * expansion.astype(np.float64)
        * reduction.astype(np.float64)
        * ln_gamma.astype(np.float64)[None, :]
    )
    # [128, CT*H]: col ct*H+h <- rho[h, ct*128 + p]
    rho_t = np.zeros((128, CT * H), np.float32)
    for ct in range(CT):
        for h in range(H):
            rho_t[:, ct * H + h] = rho[h, ct * 128 : (ct + 1) * 128]
    return a, q, W, rho_t


def _beta_term(ln_beta, expansion, reduction, a, q):
    if not np.any(ln_beta):
        return None
    # EMA of the constant beta*e_h channel: C_h[n] = a(1-q^{n+1})/(1-q)
    n_idx = np.arange(N, dtype=np.float64)
    Cn = a[:, None] * (1.0 - q[:, None] ** (n_idx[None, :] + 1.0)) / (1.0 - q[:, None])
    w = (
        expansion.astype(np.float64)
        * reduction.astype(np.float64)
        * ln_beta.astype(np.float64)[None, :]
    )  # [H, C]
    return np.einsum("hc,hn->cn", w, Cn).astype(np.float32)  # [C, N]


def _make_in_maps(x, W, rho_t):
    NW = NHALF + W
    in_maps = []
    for core in range(N_CORES):
        b, half = divmod(core, 2)
        xs = np.zeros((C, NW), np.float32)
        s = half * NHALF - W
        if s < 0:
            xs[:, W:] = x[b, :, :NHALF]
        else:
            xs[:] = x[b, :, s : s + NW]
        in_maps.append({"xs": xs, "rho": rho_t})
    return in_maps


def kernel(x, ln_gamma, ln_beta, expansion, reduction, alphas, dampen_factors,
           trace=False):
    _install_ntff_shim()
    from concourse.bass_utils import run_bass_kernel_spmd
    from concourse.bass_interp import get_hw_module

    x = np.asarray(x, np.float32)
    a, q, W, rho_t = _host_params(
        np.asarray(ln_gamma), np.asarray(ln_beta), np.asarray(expansion),
        np.asarray(reduction), np.asarray(alphas), np.asarray(dampen_factors),
    )
    nc = build_program(q, W)
    _split_multiwait(nc)
    nc.m = get_hw_module(nc.m)

    in_maps = _make_in_maps(x, W, rho_t)
    res = run_bass_kernel_spmd(
        nc, in_maps, core_ids=list(range(N_CORES)), trace=trace
    )

    out = np.empty((B, C, N), np.float32)
    for core in range(N_CORES):
        b, half = divmod(core, 2)
        out[b, :, half * NHALF : (half + 1) * NHALF] = res.results[core]["out_t"]
    bt = _beta_term(
        np.asarray(ln_beta), np.asarray(expansion), np.asarray(reduction), a, q
    )
    if bt is not None:
        out += bt[None]
    if trace:
        kernel.last_results = res
    return out


# revision 15
# speedup vs baseline: 7.4200x; 7.4200x over previous
"""MultiHeadEMABlock Trainium2 kernel (8-core SPMD, bass/Tile).

Math (reference):
  h = LayerNorm_c(x[b,c,n] over c) * gamma + beta          (per (b,n))
  xe[b,n,h,d] = h[b,n,d] * expansion[h,d]
  y = causal damped EMA along n: y[t] = a_h*sum_{s<=t} q_h^{t-s} xe[s]
  out[b,d,n] = sum_h y[b,n,h,d]*reduction[h,d] + x

Identities used:
  - Per-(h,d) scales commute with the EMA (it mixes along n only):
      out = x + sum_h rho_h[d] * S_h[d,n],  rho_h[d] = a_h*e[h,d]*r[h,d]*gamma[d]
      S_h = EMA(q_h) applied to the normalized input z.
  - beta contributes a data-independent low-rank term added on host (exact).

Sharding: 8 cores = 4 batches x 2 sequence halves. Each core processes its
half plus a W-column halo from the left (zero-padded for the first half);
q^W underflows, so results are exact without any cross-core collective.

Device algorithm (per core, c-major [channel x n] base layout):
  1. LayerNorm stats via replicated ones-matmuls on PE; z = (x-m)*rstd (DVE),
     rstd = exp(-0.5*ln(var+eps)) on ACT (Rsqrt table is unusable here).
  2. EMA as chunked causal convolution on PE, chunk L=128:
     - scale+transpose fused: one matmul per (chunk,dtile,headgroup) with a
       diag(rho_h) packed rhs turns c-major z into n-major per-head scaled
       inputs X_h (4 heads per N=512 matmul).
     - per chunk, 8 lower-triangular T_h matmuls head-accumulate in PSUM,
       plus a K=8 rank-8 carry-correction matmul (q_h^{i+1} profiles).
     - carries tracked per head via an unscaled transpose + end-row matmul
       (E), propagated with tiny [8,512] DVE ops.
  3. Back-transpose to c-major via identity matmuls, residual add on GpSimd,
     DMA out.
"""
import contextlib
import ctypes
import sys
import types

import numpy as np

for _p in ("/root/.axon_site/_ro/trn_rl_repo", "/opt/trn_rl_repo"):
    if _p not in sys.path:
        sys.path.append(_p)

B, C, N, H = 4, 512, 4096, 8
EPS = 1e-5
N_CORES = 8
NHALF = N // 2
CT = C // 128  # channel tiles
L = 128  # EMA chunk length


# ---------------------------------------------------------------------------
# axon NTFF shim (lets run_bass_kernel_spmd(trace=True) capture HW profiles)
# ---------------------------------------------------------------------------
def _install_ntff_shim():
    if "antenv.axon_hooks" in sys.modules:
        return
    holder = {"hook": None}

    def _make(so_path):
        try:
            lib = ctypes.CDLL(so_path)
        except OSError:
            return None
        if not hasattr(lib, "axon_start_nrt_profile"):
            return None
        lib.axon_start_nrt_profile.argtypes = [
            ctypes.POINTER(ctypes.c_int64),
            ctypes.c_size_t,
        ]
        lib.axon_start_nrt_profile.restype = ctypes.c_int64
        lib.axon_stop_nrt_profile.argtypes = [ctypes.c_char_p]
        lib.axon_stop_nrt_profile.restype = ctypes.c_int64

        @contextlib.contextmanager
        def _hook(output_dir, device_ids):
            import jax

            jax.devices()
            if device_ids:
                ids = (ctypes.c_int64 * len(device_ids))(*device_ids)
                rc = lib.axon_start_nrt_profile(ids, len(device_ids))
            else:
                rc = lib.axon_start_nrt_profile(None, 0)
            if rc != 0:
                raise RuntimeError(f"axon_start_nrt_profile rc={rc}")
            try:
                yield
            finally:
                n = lib.axon_stop_nrt_profile(str(output_dir).encode())
                print(f"ntff profile: {n} file(s) -> {output_dir}", file=sys.stderr)

        return _hook

    mod = types.ModuleType("antenv.axon_hooks")
    mod.set_axon_ntff_profile_hook = lambda h: holder.__setitem__("hook", h)
    mod.get_axon_ntff_profile_hook = lambda: holder["hook"]
    sys.modules["antenv.axon_hooks"] = mod
    try:
        import antenv

        antenv.axon_hooks = mod
    except ImportError:
        pass
    holder["hook"] = _make("/opt/axon/libaxon_pjrt.so")


def _split_multiwait(nc, max_waits=1):
    """This walrus build rejects >1 sync wait per instruction; split extras
    onto same-engine NoOps inserted just before (per-engine order is the
    execution order, so semantics are preserved)."""
    from concourse import mybir

    k = [0]
    for fn in nc.m.functions:
        for blk in fn.blocks:
            out = []
            for inst in blk.instructions:
                si = getattr(inst, "sync_info", None)
                if si is not None and len(si.on_wait) > max_waits:
                    waits = list(si.on_wait)
                    for w in waits[max_waits:]:
                        k[0] += 1
                        out.append(
                            mybir.InstNoOp(
                                name=f"{inst.name}-mw{k[0]}",
                                sync_info=mybir.SyncInfo(on_wait=[w], on_update=[]),
                                bass_nofuse=True,
                                engine=inst.engine,
                            )
                        )
                    inst.sync_info = mybir.SyncInfo(
                        on_wait=waits[:max_waits], on_update=list(si.on_update)
                    )
                out.append(inst)
            blk.instructions[:] = out


# ---------------------------------------------------------------------------
# program builder
# ---------------------------------------------------------------------------
def build_program(W):
    """Build the SPMD per-core program. W: halo width (multiple of L)."""
    import concourse.bass as bass
    import concourse.tile as tile
    from concourse import mybir

    NW = NHALF + W
    K0 = W // L
    NCH = NW // L  # chunks
    # ragged 512-wide stat chunks
    stat_slices = []
    o = 0
    while o < NW:
        w = min(512, NW - o)
        stat_slices.append((o, w))
        o += w
    f32 = mybir.dt.float32
    bf16 = mybir.dt.bfloat16
    Op = mybir.AluOpType
    Act = mybir.ActivationFunctionType

    nc = bass.Bass(
        "TRN2",
        target_bir_lowering=False,
        debug=False,
        enable_asserts=False,
        num_devices=N_CORES,
    )
    xs_d = nc.dram_tensor("xs", [C, NW], f32, kind="ExternalInput").ap()
    tm_d = nc.dram_tensor("tmats", [H * 128, 128], bf16, kind="ExternalInput").ap()
    w4_d = nc.dram_tensor("w4", [H * 128, 512], bf16, kind="ExternalInput").ap()
    ek_d = nc.dram_tensor("ek", [128, H], bf16, kind="ExternalInput").ap()
    pm_d = nc.dram_tensor("pmat", [H, 128], bf16, kind="ExternalInput").ap()
    id_d = nc.dram_tensor("ident", [128, 128], bf16, kind="ExternalInput").ap()
    rh_d = nc.dram_tensor("rho_hd", [H, C], f32, kind="ExternalInput").ap()
    ql_d = nc.dram_tensor("qlcol", [H, 1], f32, kind="ExternalInput").ap()
    out_d = nc.dram_tensor("out_t", [C, NHALF], f32, kind="ExternalOutput").ap()

    with tile.TileContext(nc) as tc:
        with contextlib.ExitStack() as ctx:
            pers = ctx.enter_context(tc.tile_pool(name="pers", bufs=1))
            xs_pool = ctx.enter_context(tc.tile_pool(name="xsp", bufs=2))
            sq_pool = ctx.enter_context(tc.tile_pool(name="sqp", bufs=4))
            ps_pool = ctx.enter_context(tc.tile_pool(name="ps", bufs=1, space="PSUM"))
            st_pool = ctx.enter_context(tc.tile_pool(name="stats", bufs=2))
            xh_pool = ctx.enter_context(tc.tile_pool(name="xhp", bufs=3))
            xu_pool = ctx.enter_context(tc.tile_pool(name="xup", bufs=3))
            cr_pool = ctx.enter_context(tc.tile_pool(name="crp", bufs=3))
            s_pool = ctx.enter_context(tc.tile_pool(name="sp", bufs=3))
            out_pool = ctx.enter_context(tc.tile_pool(name="outp", bufs=3))

            # ---- constants ----
            T8 = [pers.tile([128, 128], bf16, tag=f"T{h}", name=f"T{h}") for h in range(H)]
            for h in range(H):
                nc.sync.dma_start(out=T8[h][:], in_=tm_d[h * 128 : (h + 1) * 128, :])
            W4 = [pers.tile([128, 512], bf16, tag=f"W4_{i}", name=f"W4_{i}") for i in range(H)]
            for i in range(H):
                nc.sync.dma_start(out=W4[i][:], in_=w4_d[i * 128 : (i + 1) * 128, :])
            ek = pers.tile([128, H], bf16, tag="ek")
            nc.sync.dma_start(out=ek[:], in_=ek_d)
            pmat = pers.tile([H, 128], bf16, tag="pmat")
            nc.sync.dma_start(out=pmat[:], in_=pm_d)
            ident = pers.tile([128, 128], bf16, tag="ident")
            nc.sync.dma_start(out=ident[:], in_=id_d)
            rho = pers.tile([H, C], f32, tag="rho")
            nc.sync.dma_start(out=rho[:], in_=rh_d)
            qlc = pers.tile([H, 1], f32, tag="qlc")
            nc.sync.dma_start(out=qlc[:], in_=ql_d)
            epsb = pers.tile([128, 1], f32, tag="eps")
            nc.gpsimd.memset(epsb[:], EPS)
            ones = pers.tile([128, 128], bf16, tag="ones")
            nc.gpsimd.memset(ones[:], 1.0 / C)

            # ---- load, cast, square ----
            xb = pers.tile([128, CT * NW], bf16, tag="xb")
            z = pers.tile([128, CT * NW], bf16, tag="z")
            xsq = []
            for ct in range(CT):
                xst = xs_pool.tile([128, NW], f32, tag="xs")
                nc.sync.dma_start(out=xst[:], in_=xs_d[ct * 128 : (ct + 1) * 128, :])
                nc.vector.tensor_scalar(
                    out=xb[:, ct * NW : (ct + 1) * NW], in0=xst[:],
                    scalar1=1.0, scalar2=None, op0=Op.mult,
                )
                sqt = sq_pool.tile([128, NW], bf16, tag="sq")
                nc.scalar.square(out=sqt[:], in_=xst[:])
                xsq.append(sqt)

            # ---- layernorm stats + z ----
            for o, wd in stat_slices:
                ps_m = ps_pool.tile([128, 512], f32, tag="ema", bufs=2)
                ps_s = ps_pool.tile([128, 512], f32, tag="ema", bufs=2)
                for ct in range(CT):
                    nc.tensor.matmul(
                        out=ps_m[:, :wd], lhsT=ones[:],
                        rhs=xb[:, ct * NW + o : ct * NW + o + wd],
                        start=(ct == 0), stop=(ct == CT - 1),
                    )
                for ct in range(CT):
                    nc.tensor.matmul(
                        out=ps_s[:, :wd], lhsT=ones[:], rhs=xsq[ct][:, o : o + wd],
                        start=(ct == 0), stop=(ct == CT - 1),
                    )
                mean_bf = st_pool.tile([128, 512], bf16, tag="meanbf")
                nc.scalar.activation(out=mean_bf[:, :wd], in_=ps_m[:, :wd], func=Act.Copy)
                m2 = st_pool.tile([128, 512], f32, tag="m2")
                nc.scalar.square(out=m2[:, :wd], in_=ps_m[:, :wd])
                var = st_pool.tile([128, 512], f32, tag="var")
                nc.vector.scalar_tensor_tensor(
                    out=var[:, :wd], in0=ps_s[:, :wd], scalar=0.0, in1=m2[:, :wd],
                    op0=Op.bypass, op1=Op.subtract,
                )
                lnv = st_pool.tile([128, 512], f32, tag="lnv")
                nc.scalar.activation(out=lnv[:, :wd], in_=var[:, :wd], func=Act.Ln, bias=epsb[:])
                rstd = st_pool.tile([128, 512], bf16, tag="rstd")
                nc.scalar.activation(out=rstd[:, :wd], in_=lnv[:, :wd], func=Act.Exp, scale=-0.5)
                for ct in range(CT):
                    t = st_pool.tile([128, 512], bf16, tag="tnorm")
                    nc.vector.tensor_tensor(
                        out=t[:, :wd], in0=xb[:, ct * NW + o : ct * NW + o + wd],
                        in1=mean_bf[:, :wd], op=Op.subtract,
                    )
                    nc.vector.tensor_tensor(
                        out=z[:, ct * NW + o : ct * NW + o + wd], in0=t[:, :wd],
                        in1=rstd[:, :wd], op=Op.mult,
                    )

            # ---- EMA chunks ----
            c_cur = cr_pool.tile([H, C], f32, tag="carry")
            nc.gpsimd.memset(c_cur[:], 0.0)

            def z_slice(k, dt):
                return z[:, dt * NW + k * L : dt * NW + (k + 1) * L]

            for k in range(NCH):
                # unscaled transpose (for carries): X_u[j, d]
                xu_ps = ps_pool.tile([128, 512], f32, tag="xps", bufs=3)
                for dt in range(CT):
                    nc.tensor.matmul(
                        out=xu_ps[:, dt * 128 : (dt + 1) * 128],
                        lhsT=z_slice(k, dt), rhs=ident[:], start=True, stop=True,
                    )
                xu = xu_pool.tile([128, 512], bf16, tag="xu")
                nc.scalar.activation(out=xu[:], in_=xu_ps[:], func=Act.Copy)
                # end-row matmul: E[h, d]
                e_ps = ps_pool.tile([H, 512], f32, tag="eps", bufs=1)
                nc.tensor.matmul(out=e_ps[:], lhsT=ek[:], rhs=xu[:], start=True, stop=True)

                # carry update: c = qL*c + E  (computed below, after c_rho snapshot)
                c_rho = None
                if k >= K0:
                    # scaled transposes: X_h packed [j, h*512 + d]
                    xh = xh_pool.tile([128, H * 512], bf16, tag="xh")
                    for g in range(2):
                        for dt in range(CT):
                            sp = ps_pool.tile([128, 512], f32, tag="xps", bufs=3)
                            nc.tensor.matmul(
                                out=sp[:], lhsT=z_slice(k, dt), rhs=W4[g * CT + dt][:],
                                start=True, stop=True,
                            )
                            # sp[j, h'*128+jj] -> xh[:, (g*4+h')*512 + dt*128 + jj]
                            dst = xh[:].rearrange("p (hh d) -> p hh d", hh=H)[
                                :, g * 4 : (g + 1) * 4, dt * 128 : (dt + 1) * 128
                            ]
                            nc.scalar.activation(
                                out=dst,
                                in_=sp[:].rearrange("p (hp jj) -> p hp jj", hp=4),
                                func=Act.Copy,
                            )

                    # carry-scaled correction operand (uses carry INTO chunk k)
                    c_rho = cr_pool.tile([H, C], bf16, tag="crho")
                    nc.vector.tensor_tensor(
                        out=c_rho[:], in0=c_cur[:], in1=rho[:], op=Op.mult
                    )

                    ema_ps = ps_pool.tile([128, 512], f32, tag="ema", bufs=2)
                    for h in range(H):
                        nc.tensor.matmul(
                            out=ema_ps[:], lhsT=T8[h][:],
                            rhs=xh[:, h * 512 : (h + 1) * 512],
                            start=(h == 0), stop=False,
                        )
                    nc.tensor.matmul(
                        out=ema_ps[:], lhsT=pmat[:], rhs=c_rho[:], start=False,
                        stop=True,
                    )

                # carry update: c = qL*c + E
                c_nxt = cr_pool.tile([H, C], f32, tag="carry")
                c_tmp = cr_pool.tile([H, C], f32, tag="ctmp")
                nc.vector.tensor_scalar(
                    out=c_tmp[:], in0=c_cur[:], scalar1=qlc[:, 0:1], scalar2=None,
                    op0=Op.mult,
                )
                nc.vector.tensor_tensor(out=c_nxt[:], in0=c_tmp[:], in1=e_ps[:], op=Op.add)
                c_cur = c_nxt

                if k < K0:
                    continue
                # back-transpose + residual + store
                s_sb = s_pool.tile([128, 512], bf16, tag="ssb")
                nc.scalar.activation(out=s_sb[:], in_=ema_ps[:], func=Act.Copy)
                t_ps = ps_pool.tile([128, 512], f32, tag="tps", bufs=1)
                for dt in range(CT):
                    nc.tensor.matmul(
                        out=t_ps[:, dt * 128 : (dt + 1) * 128],
                        lhsT=s_sb[:, dt * 128 : (dt + 1) * 128], rhs=ident[:],
                        start=True, stop=True,
                    )
                o_sb = s_pool.tile([128, 512], bf16, tag="osb")
                nc.scalar.activation(out=o_sb[:], in_=t_ps[:], func=Act.Copy)
                ot = out_pool.tile([128, 512], f32, tag="out")
                resid = xb.rearrange("p (dt t) -> p dt t", dt=CT)[
                    :, :, k * L : (k + 1) * L
                ]
                nc.gpsimd.tensor_tensor(
                    out=ot[:].rearrange("p (dt i) -> p dt i", dt=CT),
                    in0=o_sb[:].rearrange("p (dt i) -> p dt i", dt=CT),
                    in1=resid, op=Op.add,
                )
                ko = k - K0
                for dt in range(CT):
                    nc.sync.dma_start(
                        out=out_d[dt * 128 : (dt + 1) * 128, ko * L : (ko + 1) * L],
                        in_=ot[:, dt * 128 : (dt + 1) * 128],
                    )
    return nc


def _host_params(ln_gamma, ln_beta, expansion, reduction, alphas, dampen_factors):
    import ml_dtypes

    a = 1.0 / (1.0 + np.exp(-alphas.astype(np.float64)))
    q = (1.0 - a) / (1.0 + np.exp(-dampen_factors.astype(np.float64)))
    qmax = float(q.max())
    W = L
    while qmax**W > 1e-12 and W < NHALF:
        W += L
    rho = (
        a[:, None]
        * expansion.astype(np.float64)
        * reduction.astype(np.float64)
        * ln_gamma.astype(np.float64)[None, :]
    )  # [H, C]
    bf = ml_dtypes.bfloat16
    ii, jj = np.meshgrid(np.arange(L), np.arange(L), indexing="ij")
    tmats = np.zeros((H * 128, 128), bf)
    for h in range(H):
        M = np.where(ii >= jj, q[h] ** np.maximum(ii - jj, 0), 0.0)  # T_h[i,j]
        tmats[h * 128 : (h + 1) * 128, :] = M.T.astype(bf)  # lhsT[j,i]
    w4 = np.zeros((H * 128, 512), bf)
    for g in range(2):
        for dt in range(CT):
            blk = np.zeros((128, 512))
            for hp in range(4):
                h = g * 4 + hp
                blk[:, hp * 128 : (hp + 1) * 128] = np.diag(rho[h, dt * 128 : (dt + 1) * 128])
            w4[(g * CT + dt) * 128 : (g * CT + dt + 1) * 128, :] = blk.astype(bf)
    ek = np.zeros((128, H), bf)
    for h in range(H):
        ek[:, h] = (q[h] ** (L - 1 - np.arange(L))).astype(bf)
    pmat = np.zeros((H, 128), bf)
    for h in range(H):
        pmat[h, :] = (q[h] ** (np.arange(L) + 1.0)).astype(bf)
    ident = np.eye(128, dtype=bf)
    rho_hd = rho.astype(np.float32)
    qlcol = (q**L).astype(np.float32).reshape(H, 1)
    consts = dict(
        tmats=tmats, w4=w4, ek=ek, pmat=pmat, ident=ident, rho_hd=rho_hd,
        qlcol=qlcol,
    )
    return a, q, W, consts


def _beta_term(ln_beta, expansion, reduction, a, q):
    if not np.any(ln_beta):
        return None
    n_idx = np.arange(N, dtype=np.float64)
    Cn = a[:, None] * (1.0 - q[:, None] ** (n_idx[None, :] + 1.0)) / (1.0 - q[:, None])
    w = (
        expansion.astype(np.float64)
        * reduction.astype(np.float64)
        * ln_beta.astype(np.float64)[None, :]
    )
    return np.einsum("hc,hn->cn", w, Cn).astype(np.float32)


def _make_in_maps(x, W, consts):
    NW = NHALF + W
    in_maps = []
    for core in range(N_CORES):
        b, half = divmod(core, 2)
        xs = np.zeros((C, NW), np.float32)
        s = half * NHALF - W
        if s < 0:
            xs[:, W:] = x[b, :, :NHALF]
        else:
            xs[:] = x[b, :, s : s + NW]
        in_maps.append(dict(consts, xs=xs))
    return in_maps


def kernel(x, ln_gamma, ln_beta, expansion, reduction, alphas, dampen_factors,
           trace=False):
    _install_ntff_shim()
    from concourse.bass_utils import run_bass_kernel_spmd
    from concourse.bass_interp import get_hw_module

    x = np.asarray(x, np.float32)
    a, q, W, consts = _host_params(
        np.asarray(ln_gamma), np.asarray(ln_beta), np.asarray(expansion),
        np.asarray(reduction), np.asarray(alphas), np.asarray(dampen_factors),
    )
    nc = build_program(W)
    _split_multiwait(nc)
    nc.m = get_hw_module(nc.m)

    in_maps = _make_in_maps(x, W, consts)
    res = run_bass_kernel_spmd(
        nc, in_maps, core_ids=list(range(N_CORES)), trace=trace
    )

    out = np.empty((B, C, N), np.float32)
    for core in range(N_CORES):
        b, half = divmod(core, 2)
        out[b, :, half * NHALF : (half + 1) * NHALF] = res.results[core]["out_t"]
    bt = _beta_term(
        np.asarray(ln_beta), np.asarray(expansion), np.asarray(reduction), a, q
    )
    if bt is not None:
        out += bt[None]
    if trace:
        kernel.last_results = res
    return out
